# revision 1
# baseline (speedup 1.0000x reference)
"""DistMult decoder on 8 Trainium2 NeuronCores.

reference: out[k, i, j] = sigmoid( sum_d x_i[i, d] * relations[k, d] * x_j[j, d] )
shapes: x_i [4096, 128] f32, x_j [4096, 128] f32, relations [8, 128] f32
output: [8, 4096, 4096] f32 (512 MiB)

Sharding: rows of x_i (N_i axis) split across the 8 cores (512 rows each);
x_j and relations replicated. Each core computes its [8, 512, 4096] slab.

The problem is output-store bound: 64 MiB of fp32 scores per core against
~358 GB/s of HBM bandwidth per core = ~190 us floor. The kernel keeps the
store pipeline saturated and hides matmul (PE) + sigmoid (ACT) under it.

Per-core pipeline:
  - inputs arrive pre-transposed ([D, N] layout, host-side np transpose) so
    the contraction dim D=128 is the SBUF partition dim for both matmul
    operands; no on-device transposes needed.
  - per relation k: scale x_i^T columns by r_k (per-partition tensor_scalar)
  - matmul in bf16 hi/lo 3-pass split (hi*hi + hi*lo + lo*hi, ~1.5e-5
    accurate, 3x faster than native fp32 matmul) or fp32r single pass
  - sigmoid on the scalar engine straight out of PSUM
  - 2 MiB DMA per [128, 4096] result block, alternating between the SP
    hardware DGE ring and the GpSimd software DGE ring
"""

import os

import numpy as np

import concourse.bass as bass
import concourse.mybir as mybir
from concourse import tile
from concourse.bass_utils import run_bass_kernel_spmd

N_I, N_J, D, K = 4096, 4096, 128, 8
N_CORES = 8
SHARD = N_I // N_CORES  # 512
P = 128
HALF = N_J // 2  # 2048
F32 = mybir.dt.float32
F32R = mybir.dt.float32r
BF16 = mybir.dt.bfloat16

# matmul input handling: "split3" = bf16 hi/lo 3-pass (fast, ~3e-5 rel err),
# "f32r" = fp32 round mode (~7e-4 rel err), "fp32" = native fp32 (exact).
MODE = os.environ.get("DISTMULT_MODE", "split3")


def _split_ctrl_waits(nc, maxw=1):
    """walrus in this container accepts only one sync-wait on several
    instruction structs (Drain/TPB_CTRL, tensor_scalar/S3D3_TS, ...); move
    excess waits onto same-engine NOPs placed immediately before. Engines
    consume their queues in order, so waiting on A (NOP) then B (inst) is
    equivalent to the inst waiting on both."""
    for f in nc.m.functions:
        for bb in f.blocks:
            newinsts = []
            for i in bb.instructions:
                si = i.sync_info
                if si is not None and len(si.on_wait) > maxw:
                    waits = list(si.on_wait)
                    extra, keep = waits[:-maxw], waits[-maxw:]
                    for idx in range(0, len(extra), maxw):
                        nop = mybir.InstNoOp(name=f"{i.name}-ws{idx}", ins=[], outs=[])
                        nop.engine = i.engine
                        nop.sync_info = mybir.SyncInfo(
                            on_wait=extra[idx : idx + maxw], on_update=[]
                        )
                        nc.register_instruction(nop)
                        newinsts.append(nop)
                    si.on_wait = keep
                newinsts.append(i)
            bb.instructions[:] = newinsts


def build(mode=MODE):
    nc = bass.Bass()
    x_iT = nc.dram_tensor("x_iT", [D, SHARD], F32, kind="ExternalInput")
    relT = nc.dram_tensor("relT", [D, K], F32, kind="ExternalInput")
    if mode == "split3":
        # duplicated first row-block of x_i^T: a 64 KB load that unblocks the
        # first matmuls ~2us before the full 256 KB x_iT load completes
        x_i0T = nc.dram_tensor("x_i0T", [D, P], F32, kind="ExternalInput")
    if mode == "split3":
        x_jT_hi = nc.dram_tensor("x_jT_hi", [D, N_J], BF16, kind="ExternalInput")
        x_jT_lo = nc.dram_tensor("x_jT_lo", [D, N_J], BF16, kind="ExternalInput")
    else:
        x_jT = nc.dram_tensor("x_jT", [D, N_J], F32R if mode == "f32r" else F32,
                              kind="ExternalInput")
    out = nc.dram_tensor("out", [K, SHARD, N_J], F32, kind="ExternalOutput")

    with tile.TileContext(nc) as tc:
        with (
            tc.tile_pool(name="const", bufs=1) as const,
            tc.tile_pool(name="w", bufs=2) as wpool,
            tc.tile_pool(name="psum", bufs=2, space=bass.MemorySpace.PSUM) as psum,
            tc.tile_pool(name="ob", bufs=4) as obuf,
            tc.tile_pool(name="obs", bufs=6) as obuf_small,
        ):
            if mode == "split3":
                # tiny duplicated loads of the first 512 rhs columns, first in
                # each ring's FIFO, so the leading 512-wide store sub-chunk
                # isn't gated on the full 256 KB rhs chunks
                xjh0a = const.tile([P, 512], BF16, tag="xjh0a")
                nc.sync.dma_start(xjh0a[:], x_jT_hi[:, 0:512])
                xjl0a = const.tile([P, 512], BF16, tag="xjl0a")
                nc.scalar.dma_start(xjl0a[:], x_jT_lo[:, 0:512])
                xi0 = const.tile([P, P], F32, tag="xi0")
                nc.sync.dma_start(xi0[:], x_i0T[:])
            rel = const.tile([P, K], F32, tag="rel")
            nc.sync.dma_start(rel[:], relT[:])
            xiT = const.tile([P, SHARD], F32, tag="xiT")
            nc.scalar.dma_start(xiT[:], x_iT[:])

            # rhs chunks per 2048-wide half; loads alternate HWDGE rings so
            # the first half lands as early as possible.
            if mode == "split3":
                rh, rl = [], []
                for s in range(4):
                    t = const.tile([P, 1024], BF16, tag=f"xjh{s}")
                    nc.sync.dma_start(t[:], x_jT_hi[:, s * 1024 : (s + 1) * 1024])
                    rh.append(t)
                    t = const.tile([P, 1024], BF16, tag=f"xjl{s}")
                    nc.scalar.dma_start(t[:], x_jT_lo[:, s * 1024 : (s + 1) * 1024])
                    rl.append(t)
            else:
                dt = F32R if mode == "f32r" else F32
                rj = []
                for h in range(2):
                    t = const.tile([P, HALF], dt, tag=f"xj{h}")
                    eng = nc.sync if h == 0 else nc.scalar
                    eng.dma_start(t[:], x_jT[:, h * HALF : (h + 1) * HALF])
                    rj.append(t)


            # warm up the sigmoid spline tables (~2.7us) under the input DMAs
            scratch = const.tile([P, 1], F32, tag="scratch")
            nc.gpsimd.memset(scratch[:], 0.0)
            nc.scalar.activation(
                scratch[:], scratch[:], mybir.ActivationFunctionType.Sigmoid
            )

            # warm up the PE clock (HAM un-throttles after ~3.4us of sustained
            # matmul activity) with dummy matmuls while the inputs stream in;
            # otherwise the first ~30us of real matmuls run at 1.2 GHz and
            # the store pipeline ramps slowly.
            wmup = const.tile([P, 512], BF16, tag="wmup")
            nc.gpsimd.memset(wmup[:], 0.0)
            wps = psum.tile([P, HALF], F32, tag="ps")
            for r in range(10):
                nc.tensor.matmul(
                    wps[:, (r % 4) * 512 : (r % 4 + 1) * 512],
                    wmup[:, 0:P],
                    wmup[:],
                    start=True,
                    stop=True,
                )
            # reader keeps the warmup matmuls live through any dead-code pass
            nc.scalar.activation(
                scratch[:], wps[:, 0:1], mybir.ActivationFunctionType.Sigmoid
            )

            if mode == "split3":
                # fast-path k=0 weights for the first 128-row block only:
                # three short DVE ops instead of the full 512-wide chain, so
                # the first matmul triplet is ready ~2us earlier
                wk0 = const.tile([P, P], F32, tag="wk0")
                nc.vector.tensor_scalar_mul(wk0[:], xi0[:], rel[:, 0:1])
                wk0_hi = const.tile([P, P], BF16, tag="wk0_hi")
                nc.vector.tensor_copy(wk0_hi[:], wk0[:])
                wk0_lo = const.tile([P, P], BF16, tag="wk0_lo")
                nc.vector.tensor_sub(wk0_lo[:], wk0[:], wk0_hi[:])

            chunk = 0
            for k in range(K):
                if mode == "split3":
                    wk = wpool.tile([P, SHARD], F32, tag="wk")
                    nc.vector.tensor_scalar_mul(wk[:], xiT[:], rel[:, k : k + 1])
                    wk_hi = wpool.tile([P, SHARD], BF16, tag="wk_hi")
                    nc.vector.tensor_copy(wk_hi[:], wk[:])
                    wk_lo = wpool.tile([P, SHARD], BF16, tag="wk_lo")
                    nc.vector.tensor_sub(wk_lo[:], wk[:], wk_hi[:])
                elif mode == "f32r":
                    wk = wpool.tile([P, SHARD], F32R, tag="wk")
                    nc.vector.tensor_scalar_mul(wk[:], xiT[:], rel[:, k : k + 1])
                else:
                    wk = wpool.tile([P, SHARD], F32, tag="wk")
                    nc.vector.tensor_scalar_mul(wk[:], xiT[:], rel[:, k : k + 1])

                for m in range(SHARD // P):  # 4 row blocks of 128
                    mc = slice(m * P, (m + 1) * P)
                    if mode == "split3" and k == 0 and m == 0:
                        # extra-fine first block: a leading 512-wide sub-chunk
                        # fed from the tiny duplicated loads, then 0.25/0.5 MiB
                        # sub-chunks, so the store stream starts while the PE
                        # is still ramping
                        subs = [
                            (0, 512, xjh0a, xjl0a, 0),
                            (512, 512, rh[0], rl[0], 512),
                            (1024, 1024, rh[1], rl[1], 0),
                            (2048, 1024, rh[2], rl[2], 0),
                            (3072, 1024, rh[3], rl[3], 0),
                        ]
                        for c0, w, th, tl, off in subs:
                            psq = psum.tile([P, w], F32, tag="ps")
                            for n2 in range(w // 512):
                                psl = psq[:, n2 * 512 : (n2 + 1) * 512]
                                rsl = slice(off + n2 * 512, off + (n2 + 1) * 512)
                                nc.tensor.matmul(
                                    psl, wk0_hi[:], th[:, rsl],
                                    start=True, stop=False,
                                )
                                nc.tensor.matmul(
                                    psl, wk0_hi[:], tl[:, rsl],
                                    start=False, stop=False,
                                )
                                nc.tensor.matmul(
                                    psl, wk0_lo[:], th[:, rsl],
                                    start=False, stop=True,
                                )
                            obq = obuf_small.tile([P, w], F32, tag="obs")
                            nc.scalar.activation(
                                obq[:], psq[:], mybir.ActivationFunctionType.Sigmoid
                            )
                            eng = nc.sync if chunk % 2 == 0 else nc.gpsimd
                            eng.dma_start(out[0, 0:P, c0 : c0 + w], obq[:])
                            chunk += 1
                        continue
                    # 1 MiB store granularity for the last block (shorter
                    # drain); 2 MiB blocks elsewhere (fewer sems, shorter
                    # kernel-tail sem-clear storm).
                    fine = k == K - 1 and m == SHARD // P - 1
                    ob = None if fine else obuf.tile([P, N_J], F32, tag="ob")
                    for h in range(2):  # two 2048-wide PSUM tiles per block
                        ps = psum.tile([P, HALF], F32, tag="ps")
                        for n4 in range(4):  # one 512-wide matmul per bank
                            cs = slice(n4 * 512, (n4 + 1) * 512)
                            psl = ps[:, cs]
                            if mode == "split3":
                                gc = h * HALF + n4 * 512
                                rsl = slice(gc % 1024, gc % 1024 + 512)
                                w_hi = (wk0_hi[:], wk_hi[:, mc])[0 if (k == 0 and m == 0) else 1]
                                w_lo = (wk0_lo[:], wk_lo[:, mc])[0 if (k == 0 and m == 0) else 1]
                                nc.tensor.matmul(
                                    psl, w_hi, rh[gc // 1024][:, rsl],
                                    start=True, stop=False,
                                )
                                nc.tensor.matmul(
                                    psl, w_hi, rl[gc // 1024][:, rsl],
                                    start=False, stop=False,
                                )
                                nc.tensor.matmul(
                                    psl, w_lo, rh[gc // 1024][:, rsl],
                                    start=False, stop=True,
                                )
                            else:
                                nc.tensor.matmul(
                                    psl, wk[:, mc], rj[h][:, cs],
                                    start=True, stop=True,
                                )
                        if fine:
                            if h == 0:
                                obh = obuf_small.tile([P, HALF], F32, tag="obs")
                                nc.scalar.activation(
                                    obh[:], ps[:],
                                    mybir.ActivationFunctionType.Sigmoid,
                                )
                                nc.sync.dma_start(out[k, mc, 0:HALF], obh[:])
                            else:
                                # taper the very last stores (1024+512+512) so
                                # the kernel-final DMA is only 0.25 MiB of
                                # data + receipt before the drain
                                for o0, w, eng in (
                                    (0, 1024, nc.scalar),
                                    (1024, 512, nc.sync),
                                    (1536, 512, nc.scalar),
                                ):
                                    obt = obuf_small.tile([P, w], F32, tag="obs")
                                    nc.scalar.activation(
                                        obt[:], ps[:, o0 : o0 + w],
                                        mybir.ActivationFunctionType.Sigmoid,
                                    )
                                    eng.dma_start(
                                        out[k, mc, HALF + o0 : HALF + o0 + w],
                                        obt[:],
                                    )
                            chunk += 1
                        else:
                            nc.scalar.activation(
                                ob[:, h * HALF : (h + 1) * HALF],
                                ps[:],
                                mybir.ActivationFunctionType.Sigmoid,
                            )
                    if not fine:
                        eng = nc.sync if chunk % 2 == 0 else nc.gpsimd
                        eng.dma_start(out[k, mc, :], ob[:])
                        chunk += 1

    _split_ctrl_waits(nc)
    return nc


_cache = {}


def kernel(x_i, x_j, relations):
    x_i = np.asarray(x_i, dtype=np.float32)
    x_j = np.asarray(x_j, dtype=np.float32)
    relations = np.asarray(relations, dtype=np.float32)
    assert x_i.shape == (N_I, D) and x_j.shape == (N_J, D)
    assert relations.shape == (K, D)

    if MODE not in _cache:
        _cache[MODE] = build(MODE)
    nc = _cache[MODE]

    x_jT = np.ascontiguousarray(x_j.T)
    relT = np.ascontiguousarray(relations.T)
    common = {"relT": relT}
    if MODE == "split3":
        import ml_dtypes

        hi = x_jT.astype(ml_dtypes.bfloat16)
        lo = (x_jT - hi.astype(np.float32)).astype(ml_dtypes.bfloat16)
        common["x_jT_hi"] = hi
        common["x_jT_lo"] = lo
    else:
        common["x_jT"] = x_jT

    in_maps = []
    for c in range(N_CORES):
        shard = np.ascontiguousarray(x_i[c * SHARD : (c + 1) * SHARD, :].T)
        m = {"x_iT": shard, **common}
        if MODE == "split3":
            m["x_i0T"] = np.ascontiguousarray(shard[:, 0:P])
        in_maps.append(m)

    trace = bool(int(os.environ.get("DISTMULT_TRACE", "0")))
    res = run_bass_kernel_spmd(nc, in_maps, list(range(N_CORES)), trace=trace)
    if trace:
        kernel.last_exec_time_ns = res.exec_time_ns
        kernel.last_results = res
    return np.concatenate([res.results[c]["out"] for c in range(N_CORES)], axis=1)



# revision 2
# speedup vs baseline: 1.5260x; 1.5260x over previous
"""DistMult decoder on 8 Trainium2 NeuronCores.

reference: out[k, i, j] = sigmoid( sum_d x_i[i, d] * relations[k, d] * x_j[j, d] )
shapes: x_i [4096, 128] f32, x_j [4096, 128] f32, relations [8, 128] f32
output: [8, 4096, 4096] f32 (512 MiB)

Sharding: rows of x_i (N_i axis) split across the 8 cores (512 rows each);
x_j and relations replicated. Each core computes its [8, 512, 4096] slab.

The tolerance (rel err < 2e-2) allows storing the sigmoid output as fp16
(quantization error ~2e-4) and computing the scores from bf16-rounded
operands (single-pass error ~1.1e-2, two-pass ~8e-3), so:

  - stores are fp16: 32 MiB per core (vs 64 MiB fp32), ~94 us of HBM time
  - matmul is a single bf16 pass (~57 us of PE time; "h2" adds a lhs
    lo-residual pass for 2x the PE time and 1.35x the accuracy margin)
  - the bottleneck is the ACT engine: sigmoid at 1 elem/cycle/lane
    @1.2 GHz over 16.8M elements/core = ~110 us, reading the f32 scores
    straight out of PSUM and writing fp16 to SBUF

Per-core pipeline:
  - inputs arrive pre-transposed ([D, N] layout, host-side np transpose) so
    the contraction dim D=128 is the SBUF partition dim for both matmul
    operands; no on-device transposes needed. x_j^T arrives pre-rounded
    to bf16 from the host.
  - per relation k: scale x_i^T columns by r_k straight to bf16
    (per-partition tensor_scalar with casting output)
  - matmul per 512-wide PSUM bank chunk, psum tiles [128, 2048] x 2
  - sigmoid on the scalar engine out of PSUM, fp16 into SBUF
  - 1 MiB DMA per [128, 4096] fp16 result block, alternating between the
    SP hardware DGE ring and the GpSimd software DGE ring
  - host upcasts the returned fp16 slabs to f32
"""

import os

import numpy as np

import concourse.bass as bass
import concourse.mybir as mybir
from concourse import tile
from concourse.bass_utils import run_bass_kernel_spmd

N_I, N_J, D, K = 4096, 4096, 128, 8
N_CORES = 8
SHARD = N_I // N_CORES  # 512
P = 128
HALF = N_J // 2  # 2048
F32 = mybir.dt.float32
F16 = mybir.dt.float16
BF16 = mybir.dt.bfloat16

# matmul passes: "h1" = single bf16 pass (~1.1e-2 rel err), "h2" = lhs
# hi/lo split, 2 passes (~8e-3 rel err). Output is always fp16.
MODE = os.environ.get("DISTMULT_MODE", "h1")


def _split_ctrl_waits(nc, maxw=1):
    """walrus in this container accepts only one sync-wait on several
    instruction structs (Drain/TPB_CTRL, tensor_scalar/S3D3_TS, ...); move
    excess waits onto same-engine NOPs placed immediately before. Engines
    consume their queues in order, so waiting on A (NOP) then B (inst) is
    equivalent to the inst waiting on both."""
    for f in nc.m.functions:
        for bb in f.blocks:
            newinsts = []
            for i in bb.instructions:
                si = i.sync_info
                if si is not None and len(si.on_wait) > maxw:
                    waits = list(si.on_wait)
                    extra, keep = waits[:-maxw], waits[-maxw:]
                    for idx in range(0, len(extra), maxw):
                        nop = mybir.InstNoOp(name=f"{i.name}-ws{idx}", ins=[], outs=[])
                        nop.engine = i.engine
                        nop.sync_info = mybir.SyncInfo(
                            on_wait=extra[idx : idx + maxw], on_update=[]
                        )
                        nc.register_instruction(nop)
                        newinsts.append(nop)
                    si.on_wait = keep
                newinsts.append(i)
            bb.instructions[:] = newinsts


def build(mode=MODE):
    nc = bass.Bass()
    x_iT = nc.dram_tensor("x_iT", [D, SHARD], F32, kind="ExternalInput")
    relT = nc.dram_tensor("relT", [D, K], F32, kind="ExternalInput")
    # duplicated first row-block of x_i^T: a 64 KB load that unblocks the
    # first matmuls before the full 256 KB x_iT load completes
    x_i0T = nc.dram_tensor("x_i0T", [D, P], F32, kind="ExternalInput")
    x_jT_hi = nc.dram_tensor("x_jT_hi", [D, N_J], BF16, kind="ExternalInput")
    out = nc.dram_tensor("out", [K, SHARD, N_J], F16, kind="ExternalOutput")

    with tile.TileContext(nc) as tc:
        with (
            tc.tile_pool(name="const", bufs=1) as const,
            tc.tile_pool(name="w", bufs=2) as wpool,
            tc.tile_pool(name="psum", bufs=2, space=bass.MemorySpace.PSUM) as psum,
            tc.tile_pool(name="ob", bufs=4) as obuf,
            tc.tile_pool(name="obs", bufs=6) as obuf_small,
        ):
            # tiny duplicated load of the first 512 rhs columns, first in
            # the ring FIFO, so the leading 512-wide store sub-chunk isn't
            # gated on the full rhs chunks
            xjh0a = const.tile([P, 512], BF16, tag="xjh0a")
            nc.sync.dma_start(xjh0a[:], x_jT_hi[:, 0:512])
            xi0 = const.tile([P, P], F32, tag="xi0")
            nc.sync.dma_start(xi0[:], x_i0T[:])
            rel = const.tile([P, K], F32, tag="rel")
            nc.sync.dma_start(rel[:], relT[:])
            xiT = const.tile([P, SHARD], F32, tag="xiT")
            nc.scalar.dma_start(xiT[:], x_iT[:])

            # rhs chunks; loads alternate HWDGE rings so the first chunk
            # lands as early as possible.
            rh = []
            for s in range(4):
                t = const.tile([P, 1024], BF16, tag=f"xjh{s}")
                eng = nc.sync if s % 2 == 0 else nc.scalar
                eng.dma_start(t[:], x_jT_hi[:, s * 1024 : (s + 1) * 1024])
                rh.append(t)

            # warm up the sigmoid spline tables (~2.7us) under the input DMAs
            scratch = const.tile([P, 1], F32, tag="scratch")
            nc.gpsimd.memset(scratch[:], 0.0)
            nc.scalar.activation(
                scratch[:], scratch[:], mybir.ActivationFunctionType.Sigmoid
            )

            # warm up the PE clock (HAM un-throttles after ~3.4us of sustained
            # matmul activity) with dummy matmuls while the inputs stream in;
            # otherwise the first ~30us of real matmuls run at 1.2 GHz and
            # the pipeline ramps slowly.
            wmup = const.tile([P, 512], BF16, tag="wmup")
            nc.gpsimd.memset(wmup[:], 0.0)
            wps = psum.tile([P, HALF], F32, tag="ps")
            for r in range(10):
                nc.tensor.matmul(
                    wps[:, (r % 4) * 512 : (r % 4 + 1) * 512],
                    wmup[:, 0:P],
                    wmup[:],
                    start=True,
                    stop=True,
                )
            # reader keeps the warmup matmuls live through any dead-code pass
            nc.scalar.activation(
                scratch[:], wps[:, 0:1], mybir.ActivationFunctionType.Sigmoid
            )

            # fast-path k=0 weights for the first 128-row block only, fed
            # from the tiny xi0 load so the first matmuls start early
            wk0_hi = const.tile([P, P], BF16, tag="wk0_hi")
            nc.vector.tensor_scalar_mul(wk0_hi[:], xi0[:], rel[:, 0:1])
            if mode == "h2":
                wk0f = const.tile([P, P], F32, tag="wk0f")
                nc.vector.tensor_scalar_mul(wk0f[:], xi0[:], rel[:, 0:1])
                wk0_lo = const.tile([P, P], BF16, tag="wk0_lo")
                nc.vector.tensor_sub(wk0_lo[:], wk0f[:], wk0_hi[:])

            def mm(psl, w_hi, w_lo, rhs):
                if mode == "h2":
                    nc.tensor.matmul(psl, w_hi, rhs, start=True, stop=False)
                    nc.tensor.matmul(psl, w_lo, rhs, start=False, stop=True)
                else:
                    nc.tensor.matmul(psl, w_hi, rhs, start=True, stop=True)

            chunk = 0
            for k in range(K):
                wk_hi = wpool.tile([P, SHARD], BF16, tag="wk_hi")
                nc.vector.tensor_scalar_mul(wk_hi[:], xiT[:], rel[:, k : k + 1])
                wk_lo = None
                if mode == "h2":
                    wkf = wpool.tile([P, SHARD], F32, tag="wkf")
                    nc.vector.tensor_scalar_mul(wkf[:], xiT[:], rel[:, k : k + 1])
                    wk_lo = wpool.tile([P, SHARD], BF16, tag="wk_lo")
                    nc.vector.tensor_sub(wk_lo[:], wkf[:], wk_hi[:])

                for m in range(SHARD // P):  # 4 row blocks of 128
                    mc = slice(m * P, (m + 1) * P)
                    if k == 0 and m == 0:
                        # extra-fine first block: a leading 512-wide sub-chunk
                        # fed from the tiny duplicated loads, then larger
                        # sub-chunks, so the store stream starts while the PE
                        # is still ramping
                        subs = [
                            (0, 512, xjh0a, 0),
                            (512, 512, rh[0], 512),
                            (1024, 1024, rh[1], 0),
                            (2048, 1024, rh[2], 0),
                            (3072, 1024, rh[3], 0),
                        ]
                        for c0, w, th, off in subs:
                            psq = psum.tile([P, w], F32, tag="ps")
                            for n2 in range(w // 512):
                                psl = psq[:, n2 * 512 : (n2 + 1) * 512]
                                rsl = slice(off + n2 * 512, off + (n2 + 1) * 512)
                                mm(psl, wk0_hi[:],
                                   wk0_lo[:] if mode == "h2" else None, th[:, rsl])
                            obq = obuf_small.tile([P, w], F16, tag="obs")
                            nc.scalar.activation(
                                obq[:], psq[:], mybir.ActivationFunctionType.Sigmoid
                            )
                            eng = nc.sync if chunk % 2 == 0 else nc.gpsimd
                            eng.dma_start(out[0, 0:P, c0 : c0 + w], obq[:])
                            chunk += 1
                        continue
                    # 512 KiB store granularity for the last block (shorter
                    # drain); 1 MiB blocks elsewhere.
                    fine = k == K - 1 and m == SHARD // P - 1
                    ob = None if fine else obuf.tile([P, N_J], F16, tag="ob")
                    for h in range(2):  # two 2048-wide PSUM tiles per block
                        ps = psum.tile([P, HALF], F32, tag="ps")
                        for n4 in range(4):  # one 512-wide matmul per bank
                            cs = slice(n4 * 512, (n4 + 1) * 512)
                            gc = h * HALF + n4 * 512
                            rsl = slice(gc % 1024, gc % 1024 + 512)
                            mm(ps[:, cs], wk_hi[:, mc],
                               wk_lo[:, mc] if mode == "h2" else None,
                               rh[gc // 1024][:, rsl])
                        if fine:
                            if h == 0:
                                obh = obuf_small.tile([P, HALF], F16, tag="obs")
                                nc.scalar.activation(
                                    obh[:], ps[:],
                                    mybir.ActivationFunctionType.Sigmoid,
                                )
                                nc.sync.dma_start(out[k, mc, 0:HALF], obh[:])
                            else:
                                # taper the very last stores (1024+512+512) so
                                # the kernel-final DMA is small before the drain
                                for o0, w, eng in (
                                    (0, 1024, nc.gpsimd),
                                    (1024, 512, nc.sync),
                                    (1536, 512, nc.scalar),
                                ):
                                    obt = obuf_small.tile([P, w], F16, tag="obs")
                                    nc.scalar.activation(
                                        obt[:], ps[:, o0 : o0 + w],
                                        mybir.ActivationFunctionType.Sigmoid,
                                    )
                                    eng.dma_start(
                                        out[k, mc, HALF + o0 : HALF + o0 + w],
                                        obt[:],
                                    )
                            chunk += 1
                        else:
                            nc.scalar.activation(
                                ob[:, h * HALF : (h + 1) * HALF],
                                ps[:],
                                mybir.ActivationFunctionType.Sigmoid,
                            )
                    if not fine:
                        eng = nc.sync if chunk % 2 == 0 else nc.gpsimd
                        eng.dma_start(out[k, mc, :], ob[:])
                        chunk += 1

    _split_ctrl_waits(nc)
    return nc


_cache = {}


def kernel(x_i, x_j, relations):
    x_i = np.asarray(x_i, dtype=np.float32)
    x_j = np.asarray(x_j, dtype=np.float32)
    relations = np.asarray(relations, dtype=np.float32)
    assert x_i.shape == (N_I, D) and x_j.shape == (N_J, D)
    assert relations.shape == (K, D)

    if MODE not in _cache:
        _cache[MODE] = build(MODE)
    nc = _cache[MODE]

    import ml_dtypes

    x_jT = np.ascontiguousarray(x_j.T)
    relT = np.ascontiguousarray(relations.T)
    common = {"relT": relT, "x_jT_hi": x_jT.astype(ml_dtypes.bfloat16)}

    in_maps = []
    for c in range(N_CORES):
        shard = np.ascontiguousarray(x_i[c * SHARD : (c + 1) * SHARD, :].T)
        m = {"x_iT": shard, "x_i0T": np.ascontiguousarray(shard[:, 0:P]), **common}
        in_maps.append(m)

    trace = bool(int(os.environ.get("DISTMULT_TRACE", "0")))
    res = run_bass_kernel_spmd(nc, in_maps, list(range(N_CORES)), trace=trace)
    if trace:
        kernel.last_exec_time_ns = res.exec_time_ns
        kernel.last_results = res
    halves = [res.results[c]["out"] for c in range(N_CORES)]
    return np.concatenate(halves, axis=1).astype(np.float32)
